# revision 1
# baseline (speedup 1.0000x reference)
"""Trainium2 Bass kernel for BeatPulseTransportCore.

Reference semantics (per batch row, R=160 bins, 3 channels):
  1. inject: h[:, :5, :] += (color*amount)[:,None,:] * w[None,:,None]; clip [0,1]
  2. advect (scatter-add with SCALAR offset): uniform 2-tap shift by
     k=floor(s) with weights p*(1-f), p*f; bins outside [0, R-1) dropped
  3. diffusion: [kd, 1-2kd, kd] stencil with zero boundary
  4. fade: last 8 bins scaled by ((R-1-idx)/8)^2

Because the advection offset is a scalar, steps 2+3 compose into a single
4-tap filter with CONSTANT coefficients along the bin axis:
  out[l] = sum_{d in {k-1,k,k+1,k+2}} alpha_d * h~[l-d]
where h~ is h with invalid source bins zeroed and zero-extension outside
[0, R).  All edge behaviour is reproduced exactly by zero-padded SBUF
blocks + zeroing the invalid columns.  Fade is a small per-column multiply
on the last 8 bins.  No scatter is needed on device.

Sharding: pure data parallel over batch across 8 cores (hint followed).
"""

import numpy as np

import concourse.bass as bass
import concourse.bacc as bacc
import concourse.mybir as mybir
from concourse import tile
from concourse.bass_utils import run_bass_kernel_spmd

R = 160
C = 3
FADE_W = 8
N_CORES = 8
B_FULL = 65536

f32 = np.float32
FP = mybir.dt.float32


def host_constants(
    offset_per_frame_60hz,
    persistence_per_frame_60hz,
    diffusion01,
    dt_seconds,
    amount01,
    spread01,
):
    offset, persistence = offset_per_frame_60hz, persistence_per_frame_60hz
    """Replicate the reference's f32 scalar math on host; returns everything
    the device program needs."""
    offset = f32(offset)
    persistence = f32(persistence)
    diffusion01 = f32(diffusion01)
    dt_seconds = f32(dt_seconds)
    amount01 = f32(amount01)
    spread01 = f32(spread01)

    dt = np.clip(dt_seconds, f32(0.0), f32(0.05)).astype(f32)
    dt_scale = f32(dt * f32(60.0))
    s = f32(offset * dt_scale)
    p = f32(persistence**dt_scale)

    amount = np.clip(amount01, f32(0.0), f32(1.0)).astype(f32)
    spread = np.clip(spread01, f32(0.0), f32(1.0)).astype(f32)
    tight = f32(f32(1.0) - spread)
    w5 = np.array(
        [
            f32(f32(0.5) + f32(0.4) * tight),
            f32(f32(0.2) * spread + f32(0.05)),
            f32(f32(0.12) * spread),
            f32(f32(0.06) * spread),
            f32(f32(0.02) * spread),
        ],
        dtype=f32,
    )

    # advect geometry, exactly as the reference computes it in f32
    i_idx = np.arange(R, dtype=f32)
    new_pos = (i_idx + s).astype(f32)
    valid = (new_pos >= f32(0.0)) & (new_pos < f32(R - 1))
    left = np.clip(np.floor(new_pos).astype(np.int32), 0, R - 2)
    frac = (new_pos - left.astype(f32)).astype(f32)

    kd = f32(f32(0.15) * diffusion01)
    cc = f32(f32(1.0) - f32(2.0) * kd)

    fade = np.ones(R, dtype=f32)
    idx = np.arange(R)
    t = ((R - 1 - idx).astype(f32) / f32(FADE_W)).astype(f32)
    fade = np.where(idx >= R - FADE_W, (t * t).astype(f32), fade).astype(f32)

    out = {
        "valid": valid,
        "left": left,
        "frac": frac,
        "p": p,
        "kd": kd,
        "cc": cc,
        "fade": fade,
        "w5": w5,
        "amount": amount,
    }

    if not valid.any():
        out.update(k=0, f=f32(0.0), i_min=0, i_max=-1, deviants=[], alphas={})
        return out

    iv = np.nonzero(valid)[0]
    i_min, i_max = int(iv[0]), int(iv[-1])
    shifts = left - np.arange(R, dtype=np.int32)
    vals, counts = np.unique(shifts[valid], return_counts=True)
    k = int(vals[np.argmax(counts)])
    nondev = iv[shifts[iv] == k]
    f = f32(frac[nondev[len(nondev) // 2]])

    wl = f32(f32(f32(1.0) - f) * p)
    wr = f32(f * p)
    alphas = {
        k - 1: float(kd * wl),
        k: float(cc * wl + kd * wr),
        k + 1: float(cc * wr + kd * wl),
        k + 2: float(kd * wr),
    }

    # rows whose f32-rounded floor lands on a different integer shift;
    # corrected with a few tiny extra instructions (measure-zero case).
    deviants = []
    for i in iv[shifts[iv] != k]:
        i = int(i)
        wl_i = f32(f32(f32(1.0) - frac[i]) * p)
        wr_i = f32(frac[i] * p)
        # per-output-column coefficient delta: true minus what the uniform
        # pass already applied for source column i
        true_c = {}
        for j, wgt in ((int(left[i]), wl_i), (int(left[i]) + 1, wr_i)):
            for l, dw in ((j - 1, kd), (j, cc), (j + 1, kd)):
                if 0 <= l < R:
                    true_c[l] = true_c.get(l, 0.0) + float(wgt) * float(dw)
        assumed_c = {}
        for d, a in alphas.items():
            l = i + d
            if 0 <= l < R:
                assumed_c[l] = a
        cols = sorted(set(true_c) | set(assumed_c))
        fix = []
        for l in cols:
            delta = (true_c.get(l, 0.0) - assumed_c.get(l, 0.0)) * float(fade[l])
            if delta != 0.0:
                fix.append((l, delta))
        if fix:
            deviants.append((i, fix))

    out.update(k=k, f=f, i_min=i_min, i_max=i_max, deviants=deviants, alphas=alphas)
    return out


def build_program(n_rows, consts, Q=8, bufs=3, add_mode="dve", pe_q=2):
    """Build the single-core Bass/Tile program for a batch shard of n_rows.

    add_mode: how the 3rd tap pair is combined into the accumulator —
      "dma": SWDGE dma accumulate (off the DVE/POOL shared SBUF port)
      "dve": plain DVE tensor_tensor add
      "pool": POOL tensor_tensor add (contends with DVE port)
    pe_q: number of q-blocks per tile computed on TensorE via scaled-identity
      float32r matmuls accumulated in PSUM (0 disables the PE path).
    """
    RT = 128 * Q  # rows per tile
    assert n_rows % RT == 0
    n_tiles = n_rows // RT

    alphas = consts["alphas"]
    have_work = len(alphas) > 0
    if have_work:
        ds = sorted(alphas.keys())
        padl = 3 * max(0, max(ds))
        padr = 3 * max(0, -min(ds))
    else:
        ds, padl, padr = [], 0, 0
    BLK = padl + R * C + padr
    i_min, i_max = consts["i_min"], consts["i_max"]
    fade = consts["fade"]
    w5 = consts["w5"]
    amount = consts["amount"]
    valid = consts["valid"]

    n_taps = len(ds)
    if not have_work:
        pe_q = 0
    pe_q = min(pe_q, Q)
    nq_d = Q - pe_q  # q-blocks handled by the ACT/DVE chain

    fade_cols = R - FADE_W  # first faded bin
    fade_vec = np.tile(fade[fade_cols:].repeat(C), Q).astype(f32)  # [Q*24]
    fade_const_np = np.broadcast_to(fade_vec, (128, Q * FADE_W * C)).copy()

    # injection weights, [p, 3j+c] = amount*w[j] (masked to advect-kept bins)
    wrow_vec = np.zeros(15, dtype=f32)
    for j in range(5):
        if valid[j]:
            wrow_vec[3 * j : 3 * j + 3] = f32(amount * w5[j])
    wrow_const_np = np.broadcast_to(wrow_vec, (128, 15)).copy()

    # scaled identities for the PE path: eye[p, di*128+m] = alpha_d * (p==m)
    eye_const_np = np.zeros((128, max(n_taps, 1) * 128), dtype=f32)
    for di, dd in enumerate(ds):
        eye_const_np[np.arange(128), di * 128 + np.arange(128)] = f32(alphas[dd])

    nc = bacc.Bacc(None)
    hist = nc.dram_tensor("history", [n_rows, R, C], FP, kind="ExternalInput")
    color = nc.dram_tensor("color_rgb", [n_rows, C], FP, kind="ExternalInput")
    fade_dram = nc.dram_tensor("fade_const", [128, Q * FADE_W * C], FP, kind="ExternalInput")
    wrow_dram = nc.dram_tensor("wrow_const", [128, 15], FP, kind="ExternalInput")
    eye_dram = nc.dram_tensor(
        "eye_const", [128, max(n_taps, 1) * 128], FP, kind="ExternalInput"
    )
    out = nc.dram_tensor("out", [n_rows, R, C], FP, kind="ExternalOutput")
    FPR = mybir.dt.float32r

    mult = mybir.AluOpType.mult
    add = mybir.AluOpType.add
    amin = mybir.AluOpType.min
    amax = mybir.AluOpType.max

    with tile.TileContext(nc) as tc:
        with (
            tc.tile_pool(name="const", bufs=1) as cpool,
            tc.tile_pool(name="data", bufs=bufs) as dpool,
            tc.tile_pool(name="outp", bufs=bufs) as opool,
            tc.tile_pool(name="ps", bufs=3, space="PSUM") as pspool,
        ):
            fade_t = cpool.tile([128, Q * FADE_W * C], FP)
            nc.sync.dma_start(fade_t[:], fade_dram[:])
            wrow_t = cpool.tile([128, 15], FP)
            nc.sync.dma_start(wrow_t[:], wrow_dram[:])
            if pe_q > 0:
                eye_t = cpool.tile([128, n_taps * 128], FP)
                nc.sync.dma_start(eye_t[:], eye_dram[:])

            for t in range(n_tiles):
                r0 = t * RT
                h_t = dpool.tile([128, Q * BLK], FP)
                o_t = opool.tile([128, Q * R * C], FP)
                col_t = dpool.tile([128, Q * C], FP)

                h3 = h_t.rearrange("p (q f) -> p q f", f=BLK)
                o3 = o_t.rearrange("p (q f) -> p q f", f=R * C)

                hsrc = hist[r0 : r0 + RT].rearrange("(q p) r c -> p q (r c)", p=128)
                csrc = color[r0 : r0 + RT].rearrange("(q p) c -> p q c", p=128)

                if have_work:
                    lo, hi = 3 * i_min, 3 * (i_max + 1)
                    # load only valid source bins; zero everything else + pads
                    nc.sync.dma_start(h3[:, :, padl + lo : padl + hi], hsrc[:, :, lo:hi])
                    nc.sync.dma_start(col_t.rearrange("p (q c) -> p q c", c=C), csrc)
                    if padl + lo > 0:
                        nc.gpsimd.memset(h3[:, :, 0 : padl + lo], 0.0)
                    if padl + hi < BLK:
                        nc.gpsimd.memset(h3[:, :, padl + hi : BLK], 0.0)

                    # inject into bins 0..4 (advect-dropped bins masked in wrow):
                    # h[:, :, :5, :] += color[:, None, :] * wrow; clamp [0, 1]
                    inj_t = dpool.tile([128, Q * 15], FP)
                    inj4 = inj_t.rearrange("p (q j c) -> p q j c", j=5, c=C)
                    colb = (
                        col_t.rearrange("p (q c) -> p q c", c=C)
                        .unsqueeze(2)
                        .broadcast_to((128, Q, 5, C))
                    )
                    wrowb = (
                        wrow_t.rearrange("p (j c) -> p j c", c=C)
                        .unsqueeze(1)
                        .broadcast_to((128, Q, 5, C))
                    )
                    nc.vector.tensor_tensor(inj4, colb, wrowb, mult)
                    hinj = h3[:, :, padl : padl + 15]
                    nc.vector.tensor_tensor(
                        hinj, hinj, inj_t.rearrange("p (q f) -> p q f", f=15), add
                    )
                    nc.vector.tensor_scalar(hinj, hinj, 1.0, 0.0, amin, amax)

                    # 4-tap constant-coefficient filter, split between TensorE
                    # (scaled-identity fp32r matmuls accumulating in PSUM, for
                    # q-blocks [0, pe_q)) and an ACT/DVE chain (rest):
                    #   ACT:  o   = a0*h[s0]          ACT: tmp = a2*h[s2]
                    #   DVE:  o   = a1*h[s1] + o
                    #   DVE:  o   = a3*h[s3] + o
                    #   add:  o  += tmp
                    taps = [(d, alphas[d]) for d in ds]

                    if pe_q > 0:
                        psum_t = pspool.tile([128, pe_q * 512], FP)
                        ps3 = psum_t.rearrange("p (q f) -> p q f", f=512)
                        for di, (dd, _) in enumerate(taps):
                            lhsT = eye_t[:, di * 128 : (di + 1) * 128]
                            for qi in range(pe_q):
                                rhs = h3[:, qi, padl - 3 * dd : padl - 3 * dd + R * C]
                                nc.tensor.matmul(
                                    psum_t[:, qi * 512 : qi * 512 + R * C],
                                    lhsT,
                                    rhs,
                                    start=(di == 0),
                                    stop=(di == n_taps - 1),
                                )
                        nc.scalar.copy(o3[:, 0:pe_q, :], ps3[:, :, 0 : R * C])

                    if nq_d > 0:
                        od = o3[:, pe_q:Q, :]

                        def hsl(d):
                            return h3[:, pe_q:Q, padl - 3 * d : padl - 3 * d + R * C]

                        d0, a0 = taps[0]
                        nc.scalar.mul(od, hsl(d0), float(a0))
                        if len(taps) > 1:
                            d1, a1 = taps[1]
                            nc.vector.scalar_tensor_tensor(
                                od, hsl(d1), float(a1), od, mult, add
                            )
                        if len(taps) > 3:
                            d3, a3 = taps[3]
                            nc.vector.scalar_tensor_tensor(
                                od, hsl(d3), float(a3), od, mult, add
                            )
                        if len(taps) > 2:
                            d2, a2 = taps[2]
                            tmp_t = opool.tile([128, nq_d * R * C], FP)
                            t3 = tmp_t.rearrange("p (q f) -> p q f", f=R * C)
                            nc.scalar.mul(t3[:, :, :], hsl(d2), float(a2))
                            if add_mode == "dma":
                                nc.gpsimd.dma_start(o_t[:, pe_q * R * C :], tmp_t[:, :], accum_op=add)
                            elif add_mode == "pool":
                                nc.gpsimd.tensor_tensor(od, od, t3[:, :, :], add)
                            else:
                                nc.vector.tensor_tensor(od, od, t3[:, :, :], add)

                    # fade on the last 8 bins
                    nc.vector.tensor_tensor(
                        o3[:, :, fade_cols * C :],
                        o3[:, :, fade_cols * C :],
                        fade_t.rearrange("p (q f) -> p q f", f=FADE_W * C),
                        mult,
                    )

                    # sparse fixups for f32 rounding deviants (rarely present)
                    for i, fix in consts["deviants"]:
                        hcol = h3[:, :, padl + 3 * i : padl + 3 * (i + 1)]
                        for l, delta in fix:
                            ocol = o3[:, :, 3 * l : 3 * (l + 1)]
                            nc.vector.scalar_tensor_tensor(
                                ocol, hcol, float(delta), ocol, mult, add
                            )
                else:
                    nc.gpsimd.memset(o_t[:], 0.0)

                nc.scalar.dma_start(
                    out[r0 : r0 + RT].rearrange("(q p) r c -> p q (r c)", p=128),
                    o3[:, :, :],
                )

    nc.compile()
    const_inputs = {
        "fade_const": fade_const_np,
        "wrow_const": wrow_const_np,
        "eye_const": eye_const_np,
    }
    return nc, const_inputs


def kernel(
    history,
    color_rgb,
    offset_per_frame_60hz,
    persistence_per_frame_60hz,
    diffusion01,
    dt_seconds,
    amount01,
    spread01,
):
    history = np.asarray(history, dtype=np.float32)
    color_rgb = np.asarray(color_rgb, dtype=np.float32)
    B = history.shape[0]
    assert B % N_CORES == 0
    shard = B // N_CORES

    consts = host_constants(
        offset_per_frame_60hz,
        persistence_per_frame_60hz,
        diffusion01,
        dt_seconds,
        amount01,
        spread01,
    )

    nc, const_inputs = build_program(shard, consts, **BUILD_OVERRIDES)

    in_maps = []
    for cid in range(N_CORES):
        sl = slice(cid * shard, (cid + 1) * shard)
        in_maps.append(
            {"history": history[sl], "color_rgb": color_rgb[sl], **const_inputs}
        )

    res = run_bass_kernel_spmd(nc, in_maps, core_ids=list(range(N_CORES)), **RUN_KWARGS)
    global LAST_RESULT
    LAST_RESULT = res
    return np.concatenate([res.results[i]["out"] for i in range(N_CORES)], axis=0)


# test-harness hooks (unused when graded: defaults are plain execution)
RUN_KWARGS: dict = {}
BUILD_OVERRIDES: dict = {}
LAST_RESULT = None

